# revision 1
# baseline (speedup 1.0000x reference)
"""Trainium2 Bass kernel for nn_Attention_xxc (dense transformer attention
with hop-distance bias). Data-parallel over batch: 8 cores x 2 batches.

Layout strategy (per core):
  - Host preps transposed inputs: xT [512, 2048], WqkvT [512, 1536] (q cols
    pre-scaled by 1/sqrt(hd)), WprojT [512, 512], biasT[h] = (alpha_h *
    sum_k w_hk Hstack_k).T in bf16.
  - qkv: q,k computed TRANSPOSED ([outch, tok], bf16), v computed NATURAL
    ([tok, vch], bf16) with a ones-column appended per head (65 cols/head).
  - scores computed transposed: S.T[m, n] = k_m . q_n + bias.T  (bias folded
    in via identity-matmul PSUM accumulation), exp on ACT -> P bf16.
  - AV: out_aug.T[d(+1), n] = v_aug.T @ P ; row 64 = softmax denominator.
  - normalize: broadcast 1/denom across partitions via K=1 matmul, multiply.
  - proj: y[n, o] = outT.T @ WprojT + bproj, natural layout, DMA out.
"""
import sys

sys.path.insert(0, "/opt/trn_rl_repo")

import numpy as np
import ml_dtypes

B, N, DIM = 16, 1024, 512
H, HD, KH = 8, 64, 5
SCALE = HD ** -0.5
NCORES = 8
BPC = B // NCORES          # batches per core
TOK = BPC * N              # tokens per core = 2048

_CACHE = {}


def _build():
    import concourse.bass as bass
    import concourse.bacc as bacc
    import concourse.mybir as mybir
    from concourse.tile import TileContext

    f32 = mybir.dt.float32
    f32r = mybir.dt.float32r
    bf16 = mybir.dt.bfloat16
    EXP = mybir.ActivationFunctionType.Exp
    CPY = mybir.ActivationFunctionType.Copy
    MUL = mybir.AluOpType.mult
    ADD = mybir.AluOpType.add

    nc = bacc.Bacc()
    xT = nc.declare_dram_parameter("xT", [DIM, TOK], bf16, isOutput=False)
    wqkvT = nc.declare_dram_parameter("wqkvT", [DIM, 3 * DIM], bf16, isOutput=False)
    wprojT = nc.declare_dram_parameter("wprojT", [DIM, DIM], bf16, isOutput=False)
    bprojb = nc.declare_dram_parameter("bprojb", [128, DIM], f32, isOutput=False)
    biasT = nc.declare_dram_parameter("biasT", [H, N, N], bf16, isOutput=False)
    eye = nc.declare_dram_parameter("eye", [128, 128], bf16, isOutput=False)
    ones64 = nc.declare_dram_parameter("ones64", [1, 64], bf16, isOutput=False)
    y = nc.declare_dram_parameter("y", [TOK, DIM], f32, isOutput=True)

    NT = TOK // 128            # 16 token tiles
    VW = H * (HD + 1)          # 520: v row width with ones col per head

    with TileContext(nc) as tc:
        with (
            tc.tile_pool(name="qk", bufs=1) as QK,
            tc.tile_pool(name="vres", bufs=1) as VR,
            tc.tile_pool(name="wp", bufs=1) as WP,
            tc.tile_pool(name="outT", bufs=1) as OT,
            tc.tile_pool(name="const", bufs=1) as CONST,
        ):
            eye_t = CONST.tile([128, 128], bf16, tag="eye", name="eye")
            nc.sync.dma_start(out=eye_t[:], in_=eye[:])
            ones_t = CONST.tile([1, 64], bf16, tag="ones", name="ones")
            nc.sync.dma_start(out=ones_t[:], in_=ones64[:])
            bpb_t = CONST.tile([128, DIM], f32, tag="bpb", name="bpb")
            nc.sync.dma_start(out=bpb_t[:], in_=bprojb[:])
            wp_t = [WP.tile([128, DIM], bf16, tag=f"wp{c}", name=f"wp{c}") for c in range(4)]
            for c in range(4):
                nc.sync.dma_start(out=wp_t[c][:], in_=wprojT[c * 128:(c + 1) * 128, :])

            qk_t = [QK.tile([128, TOK], bf16, tag=f"qk{o}", name=f"qk{o}") for o in range(8)]
            v_t = [VR.tile([128, VW], bf16, tag=f"v{t}", name=f"v{t}") for t in range(NT)]
            oT_t = [OT.tile([128, N], bf16, tag=f"oT{b}_{c}", name=f"oT{b}_{c}")
                    for b in range(BPC) for c in range(4)]

            # ---------------- phase 1: qkv projections ----------------
            with (
                tc.tile_pool(name="xw", bufs=1) as XW,
                tc.tile_pool(name="ps1", bufs=4, space="PSUM") as PS1,
            ):
                xT_t = [XW.tile([128, TOK], bf16, tag=f"x{c}", name=f"x{c}") for c in range(4)]
                wq_t = [XW.tile([128, 3 * DIM], bf16, tag=f"w{c}", name=f"w{c}") for c in range(4)]
                for c in range(4):
                    nc.sync.dma_start(out=xT_t[c][:], in_=xT[c * 128:(c + 1) * 128, :])
                    nc.sync.dma_start(out=wq_t[c][:], in_=wqkvT[c * 128:(c + 1) * 128, :])

                # q,k transposed: qkvT[o_tile, tok] ; o tiles 0..7 cover q,k
                for o in range(8):
                    for t in range(4):           # tok chunks of 512
                        ps = PS1.tile([128, 512], f32, tag="ps1", name="ps1")
                        for c in range(4):
                            nc.tensor.matmul(
                                ps[:], wq_t[c][:, o * 128:(o + 1) * 128],
                                xT_t[c][:, t * 512:(t + 1) * 512],
                                start=(c == 0), stop=(c == 3))
                        nc.vector.tensor_copy(qk_t[o][:, t * 512:(t + 1) * 512], ps[:])
                # v natural: [tok_tile, vch] -> packed per head with ones col
                for t in range(NT):
                    ps = PS1.tile([128, 512], f32, tag="ps1", name="ps1")
                    for c in range(4):
                        nc.tensor.matmul(
                            ps[:], xT_t[c][:, t * 128:(t + 1) * 128],
                            wq_t[c][:, 2 * DIM:3 * DIM],
                            start=(c == 0), stop=(c == 3))
                    dst = v_t[t][:, 0:VW].rearrange("p (h s) -> p h s", s=HD + 1)
                    nc.vector.tensor_copy(
                        dst[:, :, 0:HD],
                        ps[:].rearrange("p (h s) -> p h s", s=HD))
                    nc.vector.memset(dst[:, :, HD:HD + 1], 1.0)

            # ---------------- phase 2: attention ----------------
            with (
                tc.tile_pool(name="biasp", bufs=18) as BP,
                tc.tile_pool(name="pp", bufs=14) as PP,
                tc.tile_pool(name="nrm", bufs=4) as NRM,
                tc.tile_pool(name="ysb", bufs=3) as YSB,
                tc.tile_pool(name="pss", bufs=2, space="PSUM") as PSS,
                tc.tile_pool(name="pso", bufs=1, space="PSUM") as PSO,
                tc.tile_pool(name="psm", bufs=2, space="PSUM") as PSM,
            ):
                for h in range(H):
                    qt, po = qk_t[h // 2], (h % 2) * 64
                    kt = qk_t[4 + h // 2]
                    b_tiles = []
                    for mi in range(8):
                        bt = BP.tile([128, N], bf16, tag="bias", name="bias")
                        nc.sync.dma_start(
                            out=bt[:], in_=biasT[h, mi * 128:(mi + 1) * 128, :])
                        b_tiles.append(bt)
                    for b in range(BPC):
                        t0 = b * N
                        p_tiles = []
                        for mi in range(8):
                            ps = PSS.tile([128, N], f32, tag="pss", name="pss")
                            for nchunk in range(2):
                                sl = slice(nchunk * 512, (nchunk + 1) * 512)
                                nc.tensor.matmul(
                                    ps[:, sl],
                                    kt[po:po + 64, t0 + mi * 128: t0 + (mi + 1) * 128],
                                    qt[po:po + 64, t0 + nchunk * 512: t0 + (nchunk + 1) * 512],
                                    start=True, stop=False)
                                nc.tensor.matmul(
                                    ps[:, sl], eye_t[:], b_tiles[mi][:, sl],
                                    start=False, stop=True)
                            pt = PP.tile([128, N], bf16, tag="p", name="p")
                            nc.scalar.activation(pt[:], ps[:], EXP)
                            p_tiles.append(pt)
                        pso = PSO.tile([HD + 1, N], f32, tag="pso", name="pso")
                        for mi in range(8):
                            for nchunk in range(2):
                                sl = slice(nchunk * 512, (nchunk + 1) * 512)
                                nc.tensor.matmul(
                                    pso[:, sl],
                                    v_t[b * 8 + mi][:, h * (HD + 1):(h + 1) * (HD + 1)],
                                    p_tiles[mi][:, sl],
                                    start=(mi == 0), stop=(mi == 7))
                        # denominator -> broadcast -> reciprocal -> normalize
                        d_t = NRM.tile([1, N], bf16, tag="d", name="d")
                        nc.vector.tensor_copy(d_t[:], pso[64:65, :])
                        R_t = NRM.tile([64, N], f32, tag="R", name="R")
                        for nchunk in range(2):
                            sl = slice(nchunk * 512, (nchunk + 1) * 512)
                            psr = PSM.tile([64, 512], f32, tag="psm", name="psm")
                            nc.tensor.matmul(psr[:], ones_t[:], d_t[:, sl],
                                             start=True, stop=True)
                            nc.vector.reciprocal(R_t[:, sl], psr[:])
                        nc.vector.tensor_tensor(
                            oT_t[b * 4 + h // 2][po:po + 64, :],
                            pso[0:64, :], R_t[:], MUL)
                # ---------------- phase 3: output projection ----------------
                for b in range(BPC):
                    for t in range(8):
                        psy = PSM.tile([128, 512], f32, tag="psm", name="psm")
                        for c in range(4):
                            nc.tensor.matmul(
                                psy[:],
                                oT_t[b * 4 + c][:, t * 128:(t + 1) * 128],
                                wp_t[c][:], start=(c == 0), stop=(c == 3))
                        yt = YSB.tile([128, DIM], f32, tag="y", name="y")
                        nc.vector.tensor_tensor(yt[:], psy[:], bpb_t[:], ADD)
                        nc.sync.dma_start(
                            out=y[b * N + t * 128: b * N + (t + 1) * 128, :],
                            in_=yt[:])
    nc.compile()
    return nc


def _prep_host(x, Hstack, hop_logits_attn, rel_alpha, Wqkv, Wproj, bproj):
    bf = ml_dtypes.bfloat16
    lg = hop_logits_attn - hop_logits_attn.max(-1, keepdims=True)
    w = np.exp(lg)
    w /= w.sum(-1, keepdims=True)                      # [H, KH]
    Bh = np.einsum("hk,kij->hij", w.astype(np.float32),
                   Hstack.astype(np.float32))          # [H, N, N]
    biasT = np.ascontiguousarray(
        (rel_alpha[:, None, None] * Bh).transpose(0, 2, 1)).astype(bf)
    wqkvT = np.ascontiguousarray(Wqkv.T).astype(np.float32).copy()
    wqkvT[:, :DIM] *= SCALE                            # fold q scaling
    wqkvT = wqkvT.astype(bf)
    wprojT = np.ascontiguousarray(Wproj.T).astype(bf)
    bprojb = np.tile(bproj[None, :], (128, 1)).astype(np.float32)
    eye = np.eye(128, dtype=np.float32).astype(bf)
    ones64 = np.ones((1, 64), dtype=np.float32).astype(bf)
    shared = dict(wqkvT=wqkvT, wprojT=wprojT, bprojb=bprojb,
                  biasT=biasT, eye=eye, ones64=ones64)
    in_maps = []
    for i in range(NCORES):
        xi = x[i * BPC:(i + 1) * BPC].reshape(TOK, DIM)
        xTi = np.ascontiguousarray(xi.T).astype(bf)
        in_maps.append(dict(xT=xTi, **shared))
    return in_maps


def kernel(**inputs):
    from concourse.bass_utils import run_bass_kernel_spmd

    if "nc" not in _CACHE:
        _CACHE["nc"] = _build()
    nc = _CACHE["nc"]
    in_maps = _prep_host(
        np.asarray(inputs["x"], np.float32),
        np.asarray(inputs["Hstack"], np.float32),
        np.asarray(inputs["hop_logits_attn"], np.float32),
        np.asarray(inputs["rel_alpha"], np.float32),
        np.asarray(inputs["Wqkv"], np.float32),
        np.asarray(inputs["Wproj"], np.float32),
        np.asarray(inputs["bproj"], np.float32))
    res = run_bass_kernel_spmd(nc, in_maps, list(range(NCORES))).results
    out = np.concatenate([r["y"].reshape(BPC, N, DIM) for r in res], axis=0)
    return out.astype(np.float32)



# revision 8
# speedup vs baseline: 4.5684x; 4.5684x over previous
"""Trainium2 Bass kernel for nn_Attention_xxc (dense transformer attention
with hop-distance bias). Data-parallel over batch: 8 cores x 2 batches.

Wire-traffic-minimized design: the warm end-to-end latency of this problem
is dominated by host<->device transfer over the axon tunnel (~50 MB/s), so
every shared tensor is shipped sharded 1/8-per-core and AllGathered on
device over NeuronLink; the hop-bias mixture  alpha_h * sum_k w_hk Hstack_k
is never materialized on the host - the PE folds it into the score matmuls
as  S.T = K^T Q + sum_k (c_hk I) @ Hstack_k.T  accumulated in PSUM.

Per-core layout (core c of 8):
  - xn [2048, 512] bf16: the core's own 2 batches, natural layout; the PE
    transposes it on device via identity matmuls.
  - shards (rows c/8) of: HTs flat [5120,1024] (Hstack_k transposed),
    wqkvT [512,1536] (q cols pre-scaled 1/sqrt(hd)), wprojT [512,512],
    ceye flat [5120,128] (40 scaled identities c_hk*I), eye128.
  - qkv: q,k TRANSPOSED ([outch, tok] bf16), v NATURAL with a ones column
    per head (65 cols/head) so the AV matmul also produces the softmax
    denominator in row 64.
  - output y [2048, 512] bf16, host casts to f32.
Runner: persistent jax jit of the bass_exec custom call (no per-call
retrace), donated output buffers are created on device (no host zeros).
"""
import sys

sys.path.insert(0, "/opt/trn_rl_repo")

import numpy as np
import ml_dtypes

B, N, DIM = 16, 1024, 512
H, HD, KH = 8, 64, 5
SCALE = HD ** -0.5
NCORES = 8
BPC = B // NCORES          # batches per core
TOK = BPC * N              # tokens per core = 2048
HTR = KH * N               # 5120 rows of flat transposed-Hstack
CER = H * KH * 128         # 5120 rows of flat scaled-identity stack

_CACHE = {}


def _build():
    import concourse.bass as bass
    import concourse.bacc as bacc
    import concourse.mybir as mybir
    from concourse.tile import TileContext

    f32 = mybir.dt.float32
    bf16 = mybir.dt.bfloat16
    EXP = mybir.ActivationFunctionType.Exp
    MUL = mybir.AluOpType.mult
    ADD = mybir.AluOpType.add
    BYP = mybir.AluOpType.bypass
    RG = [list(range(NCORES))]

    nc = bacc.Bacc(num_devices=NCORES)
    xn = nc.declare_dram_parameter("xn", [TOK, DIM], bf16, isOutput=False)
    hts_in = nc.declare_dram_parameter("hts_in", [HTR // 8, N], bf16, isOutput=False)
    wqkv_in = nc.declare_dram_parameter("wqkv_in", [DIM // 8, 3 * DIM], bf16, isOutput=False)
    wproj_in = nc.declare_dram_parameter("wproj_in", [DIM // 8, DIM], bf16, isOutput=False)
    ceye_in = nc.declare_dram_parameter("ceye_in", [CER // 8, 128], bf16, isOutput=False)
    eye_in = nc.declare_dram_parameter("eye_in", [16, 128], bf16, isOutput=False)
    bprojr = nc.declare_dram_parameter("bprojr", [1, DIM], f32, isOutput=False)
    y = nc.declare_dram_parameter("y", [TOK, DIM], bf16, isOutput=True)

    NT = TOK // 128            # 16 token tiles per core
    VW = H * (HD + 1)          # 520: v row width with ones col per head

    with TileContext(nc) as tc:
        with (
            tc.tile_pool(name="dram", bufs=1, space="DRAM") as DR,
            tc.tile_pool(name="qk", bufs=1) as QK,
            tc.tile_pool(name="vres", bufs=1) as VR,
            tc.tile_pool(name="wp", bufs=1) as WP,
            tc.tile_pool(name="outT", bufs=1) as OT,
            tc.tile_pool(name="const", bufs=1) as CONST,
        ):
            # ---------------- phase 0: AllGather shared tensors ----------------
            def gathered(name, inp, rows, cols, dt):
                bnc = DR.tile([rows // 8, cols], dt, tag=f"b_{name}", name=f"b_{name}")
                full = DR.tile([rows, cols], dt, tag=f"g_{name}", name=f"g_{name}")
                nc.gpsimd.dma_start(bnc[:], inp[:])
                nc.gpsimd.collective_compute(
                    "AllGather", BYP, replica_groups=RG,
                    ins=[bnc.opt()], outs=[full.opt()])
                return full

            eye_full = gathered("eye", eye_in, 128, 128, bf16)
            wqkv_full = gathered("wqkv", wqkv_in, DIM, 3 * DIM, bf16)
            wproj_full = gathered("wproj", wproj_in, DIM, DIM, bf16)
            ceye_full = gathered("ceye", ceye_in, CER, 128, bf16)
            hts_full = gathered("hts", hts_in, HTR, N, bf16)

            eye_t = CONST.tile([128, 128], bf16, tag="eye", name="eye")
            nc.sync.dma_start(out=eye_t[:], in_=eye_full[:])
            ones_t = CONST.tile([1, 128], bf16, tag="ones", name="ones")
            nc.vector.memset(ones_t[:], 1.0)
            onesf_t = CONST.tile([1, 128], f32, tag="onesf", name="onesf")
            nc.vector.memset(onesf_t[:], 1.0)
            ceye_t = CONST.tile([128, H * KH * 128], bf16, tag="ceye", name="ceye")
            for j in range(H * KH):
                nc.sync.dma_start(out=ceye_t[:, j * 128:(j + 1) * 128],
                                  in_=ceye_full[j * 128:(j + 1) * 128, :])
            wp_t = [WP.tile([128, DIM], bf16, tag=f"wp{c}", name=f"wp{c}") for c in range(4)]
            for c in range(4):
                nc.sync.dma_start(out=wp_t[c][:], in_=wproj_full[c * 128:(c + 1) * 128, :])

            qk_t = [QK.tile([128, TOK], bf16, tag=f"qk{o}", name=f"qk{o}") for o in range(8)]
            v_t = [VR.tile([128, VW], bf16, tag=f"v{t}", name=f"v{t}") for t in range(NT)]
            oT_t = [OT.tile([128, N], bf16, tag=f"oT{b}_{c}", name=f"oT{b}_{c}")
                    for b in range(BPC) for c in range(4)]

            # broadcast bproj across 128 partitions: ones^T [128] x bproj [1,512]
            bpb_t = CONST.tile([128, DIM], f32, tag="bpb", name="bpb")
            bpr_t = CONST.tile([1, DIM], f32, tag="bpr", name="bpr")
            nc.sync.dma_start(out=bpr_t[:], in_=bprojr[:])

            # ---------------- phase 1: x transpose + qkv projections ----------------
            with (
                tc.tile_pool(name="xw", bufs=1) as XW,
                tc.tile_pool(name="ps1", bufs=4, space="PSUM") as PS1,
                tc.tile_pool(name="pst", bufs=4, space="PSUM") as PST,
            ):
                psb = PS1.tile([128, DIM], f32, tag="ps1", name="ps1")
                nc.tensor.matmul(psb[:], onesf_t[:], bpr_t[:], start=True, stop=True)
                nc.vector.tensor_copy(bpb_t[:], psb[:])

                xn_t = [XW.tile([128, DIM], bf16, tag=f"xn{t}", name=f"xn{t}")
                        for t in range(NT)]
                for t in range(NT):
                    nc.sync.dma_start(out=xn_t[t][:], in_=xn[t * 128:(t + 1) * 128, :])
                xT_t = [XW.tile([128, TOK], bf16, tag=f"x{c}", name=f"x{c}") for c in range(4)]
                for t in range(NT):
                    for c in range(4):
                        pst = PST.tile([128, 128], f32, tag="pst", name="pst")
                        nc.tensor.matmul(pst[:], xn_t[t][:, c * 128:(c + 1) * 128],
                                         eye_t[:], start=True, stop=True)
                        nc.vector.tensor_copy(xT_t[c][:, t * 128:(t + 1) * 128], pst[:])

                wq_t = [XW.tile([128, 3 * DIM], bf16, tag=f"w{c}", name=f"w{c}") for c in range(4)]
                for c in range(4):
                    nc.sync.dma_start(out=wq_t[c][:], in_=wqkv_full[c * 128:(c + 1) * 128, :])

                # q,k transposed: qkvT[o_tile, tok] ; o tiles 0..7 cover q,k
                for o in range(8):
                    for t in range(4):           # tok chunks of 512
                        ps = PS1.tile([128, 512], f32, tag="ps1", name="ps1")
                        for c in range(4):
                            nc.tensor.matmul(
                                ps[:], wq_t[c][:, o * 128:(o + 1) * 128],
                                xT_t[c][:, t * 512:(t + 1) * 512],
                                start=(c == 0), stop=(c == 3))
                        nc.vector.tensor_copy(qk_t[o][:, t * 512:(t + 1) * 512], ps[:])
                # v natural: [tok_tile, vch] -> packed per head with ones col
                for t in range(NT):
                    ps = PS1.tile([128, 512], f32, tag="ps1", name="ps1")
                    for c in range(4):
                        nc.tensor.matmul(
                            ps[:], xT_t[c][:, t * 128:(t + 1) * 128],
                            wq_t[c][:, 2 * DIM:3 * DIM],
                            start=(c == 0), stop=(c == 3))
                    dst = v_t[t][:, 0:VW].rearrange("p (h s) -> p h s", s=HD + 1)
                    nc.vector.tensor_copy(
                        dst[:, :, 0:HD],
                        ps[:].rearrange("p (h s) -> p h s", s=HD))
                    nc.vector.memset(dst[:, :, HD:HD + 1], 1.0)

            # ---------------- phase 2: attention ----------------
            with (
                tc.tile_pool(name="htp", bufs=4) as HTP,
                tc.tile_pool(name="pp", bufs=18) as PP,
                tc.tile_pool(name="nrm", bufs=4) as NRM,
                tc.tile_pool(name="ysb", bufs=3) as YSB,
                tc.tile_pool(name="pss", bufs=2, space="PSUM") as PSS,
                tc.tile_pool(name="pso", bufs=1, space="PSUM") as PSO,
                tc.tile_pool(name="psm", bufs=2, space="PSUM") as PSM,
            ):
                for h in range(H):
                    qt, po = qk_t[h // 2], (h % 2) * 64
                    kt = qk_t[4 + h // 2]
                    p_tiles = [[], []]
                    for mi in range(8):
                        ht = HTP.tile([128, KH * N], bf16, tag="ht", name="ht")
                        for k in range(KH):
                            nc.sync.dma_start(
                                out=ht[:, k * N:(k + 1) * N],
                                in_=hts_full[k * N + mi * 128: k * N + (mi + 1) * 128, :])
                        for b in range(BPC):
                            t0 = b * N
                            ps = PSS.tile([128, N], f32, tag="pss", name="pss")
                            for nchunk in range(2):
                                sl = slice(nchunk * 512, (nchunk + 1) * 512)
                                nc.tensor.matmul(
                                    ps[:, sl],
                                    kt[po:po + 64, t0 + mi * 128: t0 + (mi + 1) * 128],
                                    qt[po:po + 64, t0 + nchunk * 512: t0 + (nchunk + 1) * 512],
                                    start=True, stop=False)
                                for k in range(KH):
                                    ci = (h * KH + k) * 128
                                    nc.tensor.matmul(
                                        ps[:, sl],
                                        ceye_t[:, ci:ci + 128],
                                        ht[:, k * N + nchunk * 512:
                                           k * N + (nchunk + 1) * 512],
                                        start=False, stop=(k == KH - 1))
                            pt = PP.tile([128, N], bf16, tag="p", name="p")
                            nc.scalar.activation(pt[:], ps[:], EXP)
                            p_tiles[b].append(pt)
                    for b in range(BPC):
                        pso = PSO.tile([HD + 1, N], f32, tag="pso", name="pso")
                        for mi in range(8):
                            for nchunk in range(2):
                                sl = slice(nchunk * 512, (nchunk + 1) * 512)
                                nc.tensor.matmul(
                                    pso[:, sl],
                                    v_t[b * 8 + mi][:, h * (HD + 1):(h + 1) * (HD + 1)],
                                    p_tiles[b][mi][:, sl],
                                    start=(mi == 0), stop=(mi == 7))
                        # denominator -> broadcast -> reciprocal -> normalize
                        d_t = NRM.tile([1, N], bf16, tag="d", name="d")
                        nc.vector.tensor_copy(d_t[:], pso[64:65, :])
                        R_t = NRM.tile([64, N], f32, tag="R", name="R")
                        for nchunk in range(2):
                            sl = slice(nchunk * 512, (nchunk + 1) * 512)
                            psr = PSM.tile([64, 512], f32, tag="psm", name="psm")
                            nc.tensor.matmul(psr[:], ones_t[:, 0:64], d_t[:, sl],
                                             start=True, stop=True)
                            nc.vector.reciprocal(R_t[:, sl], psr[:])
                        nc.vector.tensor_tensor(
                            oT_t[b * 4 + h // 2][po:po + 64, :],
                            pso[0:64, :], R_t[:], MUL)
                # ---------------- phase 3: output projection ----------------
                for b in range(BPC):
                    for t in range(8):
                        psy = PSM.tile([128, 512], f32, tag="psm", name="psm")
                        for c in range(4):
                            nc.tensor.matmul(
                                psy[:],
                                oT_t[b * 4 + c][:, t * 128:(t + 1) * 128],
                                wp_t[c][:], start=(c == 0), stop=(c == 3))
                        yt = YSB.tile([128, DIM], bf16, tag="y", name="y")
                        nc.vector.tensor_tensor(yt[:], psy[:], bpb_t[:], ADD)
                        nc.sync.dma_start(
                            out=y[b * N + t * 128: b * N + (t + 1) * 128, :],
                            in_=yt[:])
    nc.compile()
    return nc


def _prep_host(x, Hstack, hop_logits_attn, rel_alpha, Wqkv, Wproj, bproj):
    """Build the GLOBAL (concatenated-over-cores) input arrays directly.

    Shard layouts are chosen so that concatenating per-core shards along
    axis 0 reproduces the flat array itself - zero np.concatenate cost.
    Returns dict name -> global array of shape [8*per_core_rows, cols].
    """
    bf = ml_dtypes.bfloat16
    lg = hop_logits_attn - hop_logits_attn.max(-1, keepdims=True)
    w = np.exp(lg)
    w /= w.sum(-1, keepdims=True)                      # [H, KH]
    c_hk = (rel_alpha[:, None] * w).astype(np.float32)  # [H, KH]
    eye = np.eye(128, dtype=np.float32)
    ceye = (c_hk.reshape(H * KH, 1, 1) * eye).astype(bf).reshape(CER, 128)
    hts = np.ascontiguousarray(Hstack.transpose(0, 2, 1)).astype(bf).reshape(HTR, N)
    wqkvT = np.ascontiguousarray(Wqkv.T).astype(np.float32)
    wqkvT[:, :DIM] *= SCALE                            # fold q scaling
    wqkvT = wqkvT.astype(bf)
    wprojT = np.ascontiguousarray(Wproj.T).astype(bf)
    return {
        "xn": x.reshape(NCORES * TOK, DIM).astype(bf),
        "hts_in": hts,
        "wqkv_in": wqkvT,
        "wproj_in": wprojT,
        "ceye_in": ceye,
        "eye_in": eye.astype(bf),
        "bprojr": np.tile(bproj.astype(np.float32)[None, :], (NCORES, 1)),
    }


def _make_runner(nc):
    """Persistent-jit runner for the bass_exec custom call (the axon/PJRT
    path), so warm calls skip tracing and output zero-buffers are created
    on device instead of being shipped from the host."""
    import jax
    import jax.numpy as jnp
    from jax.sharding import Mesh, PartitionSpec, NamedSharding
    from jax.experimental.shard_map import shard_map
    from concourse import mybir
    from concourse.bass2jax import (
        _bass_exec_p, partition_id_tensor, install_neuronx_cc_hook)

    install_neuronx_cc_hook()
    partition_name = nc.partition_id_tensor.name if nc.partition_id_tensor else None
    in_names, out_names, out_avals = [], [], []
    for alloc in nc.m.functions[0].allocations:
        if not isinstance(alloc, mybir.MemoryLocationSet):
            continue
        name = alloc.memorylocations[0].name
        if alloc.kind == "ExternalInput":
            if name != partition_name:
                in_names.append(name)
        elif alloc.kind == "ExternalOutput":
            out_names.append(name)
            out_avals.append(jax.core.ShapedArray(
                tuple(alloc.tensor_shape), mybir.dt.np(alloc.dtype)))
    n_params = len(in_names)
    n_outs = len(out_avals)
    all_names = in_names + out_names
    if partition_name is not None:
        all_names = all_names + [partition_name]
    donate = tuple(range(n_params, n_params + n_outs))

    def _body(*args):
        operands = list(args)
        if partition_name is not None:
            operands.append(partition_id_tensor())
        outs = _bass_exec_p.bind(
            *operands, out_avals=tuple(out_avals), in_names=tuple(all_names),
            out_names=tuple(out_names), lowering_input_output_aliases=(),
            sim_require_finite=True, sim_require_nnan=True, nc=nc)
        return tuple(outs)

    devices = jax.devices()[:NCORES]
    mesh = Mesh(np.asarray(devices), ("core",))
    spec = NamedSharding(mesh, PartitionSpec("core"))
    in_specs = (PartitionSpec("core"),) * (n_params + n_outs)
    out_specs = (PartitionSpec("core"),) * n_outs
    sharded = jax.jit(
        shard_map(_body, mesh=mesh, in_specs=in_specs, out_specs=out_specs,
                  check_rep=False),
        donate_argnums=donate, keep_unused=True)

    zero_shapes = [(NCORES * a.shape[0], *a.shape[1:]) for a in out_avals]
    zero_dtypes = [a.dtype for a in out_avals]
    zeros_fn = jax.jit(
        lambda: tuple(jnp.zeros(s, d) for s, d in zip(zero_shapes, zero_dtypes)),
        out_shardings=tuple(spec for _ in out_avals))

    def run(global_in: dict):
        ins = [global_in[name] for name in in_names]
        zs = zeros_fn()
        outs = sharded(*ins, *zs)
        return {name: np.asarray(o) for name, o in zip(out_names, outs)}

    return run


def kernel(**inputs):
    if "run" not in _CACHE:
        _CACHE["nc"] = _build()
        _CACHE["run"] = _make_runner(_CACHE["nc"])
    gin = _prep_host(
        np.asarray(inputs["x"], np.float32),
        np.asarray(inputs["Hstack"], np.float32),
        np.asarray(inputs["hop_logits_attn"], np.float32),
        np.asarray(inputs["rel_alpha"], np.float32),
        np.asarray(inputs["Wqkv"], np.float32),
        np.asarray(inputs["Wproj"], np.float32),
        np.asarray(inputs["bproj"], np.float32))
    outs = _CACHE["run"](gin)
    return outs["y"].astype(np.float32).reshape(B, N, DIM)


# revision 15
# speedup vs baseline: 4.6435x; 1.0164x over previous
"""Trainium2 Bass kernel for nn_Attention_xxc (dense transformer attention
with hop-distance bias). Data-parallel over batch: 8 cores x 2 batches.

Wire-traffic-minimized design: the warm end-to-end latency of this problem
is dominated by host<->device transfer over the axon tunnel (~50 MB/s), so
every shared tensor is shipped sharded 1/8-per-core and AllGathered on
device over NeuronLink; the hop-bias mixture  alpha_h * sum_k w_hk Hstack_k
is never materialized on the host - the PE folds it into the score matmuls
as  S.T = K^T Q + sum_k (c_hk I) @ Hstack_k.T  accumulated in PSUM.

Per-core layout (core c of 8):
  - xn [2048, 512] bf16: the core's own 2 batches, natural layout; the PE
    transposes it on device via identity matmuls.
  - shards (rows c/8) of: HTs flat [5120,1024] (Hstack_k transposed),
    wqkvT [512,1536] (q cols pre-scaled 1/sqrt(hd)), wprojT [512,512],
    ceye flat [5120,128] (40 scaled identities c_hk*I), eye128.
  - qkv: q,k TRANSPOSED ([outch, tok] bf16), v NATURAL with a ones column
    per head (65 cols/head) so the AV matmul also produces the softmax
    denominator in row 64.
  - output y [2048, 512] bf16, host casts to f32.
Runner: persistent jax jit of the bass_exec custom call (no per-call
retrace), donated output buffers are created on device (no host zeros).
"""
import sys

sys.path.insert(0, "/opt/trn_rl_repo")

import numpy as np
import ml_dtypes

B, N, DIM = 16, 1024, 512
H, HD, KH = 8, 64, 5
SCALE = HD ** -0.5
NCORES = 8
BPC = B // NCORES          # batches per core
TOK = BPC * N              # tokens per core = 2048
HTR = KH * N               # 5120 rows of flat transposed-Hstack
CER = H * KH * 128         # 5120 rows of flat scaled-identity stack

_CACHE = {}


def _build():
    import concourse.bass as bass
    import concourse.bacc as bacc
    import concourse.mybir as mybir
    from concourse.tile import TileContext

    f32 = mybir.dt.float32
    bf16 = mybir.dt.bfloat16
    u8 = mybir.dt.uint8
    EXP = mybir.ActivationFunctionType.Exp
    MUL = mybir.AluOpType.mult
    ADD = mybir.AluOpType.add
    BYP = mybir.AluOpType.bypass
    RG = [list(range(NCORES))]

    nc = bacc.Bacc(num_devices=NCORES)
    xn = nc.declare_dram_parameter("xn", [TOK, DIM], bf16, isOutput=False)
    hts_in = nc.declare_dram_parameter("hts_in", [HTR // 8, N], u8, isOutput=False)
    wqkv_in = nc.declare_dram_parameter("wqkv_in", [DIM // 8, 3 * DIM], bf16, isOutput=False)
    wproj_in = nc.declare_dram_parameter("wproj_in", [DIM // 8, DIM], bf16, isOutput=False)
    ceye_in = nc.declare_dram_parameter("ceye_in", [CER // 8, 128], bf16, isOutput=False)
    eye_in = nc.declare_dram_parameter("eye_in", [16, 128], bf16, isOutput=False)
    bprojr = nc.declare_dram_parameter("bprojr", [1, DIM], f32, isOutput=False)
    y = nc.declare_dram_parameter("y", [TOK, DIM], bf16, isOutput=True)

    NT = TOK // 128            # 16 token tiles per core
    VW = H * (HD + 1)          # 520: v row width with ones col per head

    with TileContext(nc) as tc:
        with (
            tc.tile_pool(name="dram", bufs=1, space="DRAM") as DR,
            tc.tile_pool(name="qk", bufs=1) as QK,
            tc.tile_pool(name="vres", bufs=1) as VR,
            tc.tile_pool(name="wp", bufs=1) as WP,
            tc.tile_pool(name="outT", bufs=1) as OT,
            tc.tile_pool(name="const", bufs=1) as CONST,
        ):
            # ---------------- phase 0: AllGather shared tensors ----------------
            def gathered(name, inp, rows, cols, dt):
                bnc = DR.tile([rows // 8, cols], dt, tag=f"b_{name}", name=f"b_{name}")
                full = DR.tile([rows, cols], dt, tag=f"g_{name}", name=f"g_{name}")
                nc.gpsimd.dma_start(bnc[:], inp[:])
                nc.gpsimd.collective_compute(
                    "AllGather", BYP, replica_groups=RG,
                    ins=[bnc.opt()], outs=[full.opt()])
                return full

            eye_full = gathered("eye", eye_in, 128, 128, bf16)
            wqkv_full = gathered("wqkv", wqkv_in, DIM, 3 * DIM, bf16)
            wproj_full = gathered("wproj", wproj_in, DIM, DIM, bf16)
            ceye_full = gathered("ceye", ceye_in, CER, 128, bf16)
            hts_full = gathered("hts", hts_in, HTR, N, u8)

            eye_t = CONST.tile([128, 128], bf16, tag="eye", name="eye")
            nc.sync.dma_start(out=eye_t[:], in_=eye_full[:])
            ones_t = CONST.tile([1, 128], bf16, tag="ones", name="ones")
            nc.vector.memset(ones_t[:], 1.0)
            onesf_t = CONST.tile([1, 128], f32, tag="onesf", name="onesf")
            nc.vector.memset(onesf_t[:], 1.0)
            ceye_t = CONST.tile([128, H * KH * 128], bf16, tag="ceye", name="ceye")
            for j in range(H * KH):
                nc.sync.dma_start(out=ceye_t[:, j * 128:(j + 1) * 128],
                                  in_=ceye_full[j * 128:(j + 1) * 128, :])
            wp_t = [WP.tile([128, DIM], bf16, tag=f"wp{c}", name=f"wp{c}") for c in range(4)]
            for c in range(4):
                nc.sync.dma_start(out=wp_t[c][:], in_=wproj_full[c * 128:(c + 1) * 128, :])

            qk_t = [QK.tile([128, TOK], bf16, tag=f"qk{o}", name=f"qk{o}") for o in range(8)]
            v_t = [VR.tile([128, VW], bf16, tag=f"v{t}", name=f"v{t}") for t in range(NT)]
            oT_t = [OT.tile([128, N], bf16, tag=f"oT{b}_{c}", name=f"oT{b}_{c}")
                    for b in range(BPC) for c in range(4)]

            # broadcast bproj across 128 partitions: ones^T [128] x bproj [1,512]
            bpb_t = CONST.tile([128, DIM], f32, tag="bpb", name="bpb")
            bpr_t = CONST.tile([1, DIM], f32, tag="bpr", name="bpr")
            nc.sync.dma_start(out=bpr_t[:], in_=bprojr[:])

            # ---------------- phase 1: x transpose + qkv projections ----------------
            with (
                tc.tile_pool(name="xw", bufs=1) as XW,
                tc.tile_pool(name="ps1", bufs=4, space="PSUM") as PS1,
                tc.tile_pool(name="pst", bufs=4, space="PSUM") as PST,
            ):
                psb = PS1.tile([128, DIM], f32, tag="ps1", name="ps1")
                nc.tensor.matmul(psb[:], onesf_t[:], bpr_t[:], start=True, stop=True)
                nc.vector.tensor_copy(bpb_t[:], psb[:])

                xn_t = [XW.tile([128, DIM], bf16, tag=f"xn{t}", name=f"xn{t}")
                        for t in range(NT)]
                for t in range(NT):
                    nc.sync.dma_start(out=xn_t[t][:], in_=xn[t * 128:(t + 1) * 128, :])
                xT_t = [XW.tile([128, TOK], bf16, tag=f"x{c}", name=f"x{c}") for c in range(4)]
                for t in range(NT):
                    for c in range(4):
                        pst = PST.tile([128, 128], f32, tag="pst", name="pst")
                        nc.tensor.matmul(pst[:], xn_t[t][:, c * 128:(c + 1) * 128],
                                         eye_t[:], start=True, stop=True)
                        nc.vector.tensor_copy(xT_t[c][:, t * 128:(t + 1) * 128], pst[:])

                wq_t = [XW.tile([128, 3 * DIM], bf16, tag=f"w{c}", name=f"w{c}") for c in range(4)]
                for c in range(4):
                    nc.sync.dma_start(out=wq_t[c][:], in_=wqkv_full[c * 128:(c + 1) * 128, :])

                # q,k transposed: qkvT[o_tile, tok] ; o tiles 0..7 cover q,k
                for o in range(8):
                    for t in range(4):           # tok chunks of 512
                        ps = PS1.tile([128, 512], f32, tag="ps1", name="ps1")
                        for c in range(4):
                            nc.tensor.matmul(
                                ps[:], wq_t[c][:, o * 128:(o + 1) * 128],
                                xT_t[c][:, t * 512:(t + 1) * 512],
                                start=(c == 0), stop=(c == 3))
                        nc.vector.tensor_copy(qk_t[o][:, t * 512:(t + 1) * 512], ps[:])
                # v natural: [tok_tile, vch] -> packed per head with ones col
                for t in range(NT):
                    ps = PS1.tile([128, 512], f32, tag="ps1", name="ps1")
                    for c in range(4):
                        nc.tensor.matmul(
                            ps[:], xT_t[c][:, t * 128:(t + 1) * 128],
                            wq_t[c][:, 2 * DIM:3 * DIM],
                            start=(c == 0), stop=(c == 3))
                    dst = v_t[t][:, 0:VW].rearrange("p (h s) -> p h s", s=HD + 1)
                    nc.vector.tensor_copy(
                        dst[:, :, 0:HD],
                        ps[:].rearrange("p (h s) -> p h s", s=HD))
                    nc.vector.memset(dst[:, :, HD:HD + 1], 1.0)

            # ---------------- phase 2: attention ----------------
            with (
                tc.tile_pool(name="htu", bufs=3) as HTU,
                tc.tile_pool(name="htp", bufs=4) as HTP,
                tc.tile_pool(name="pp", bufs=18) as PP,
                tc.tile_pool(name="nrm", bufs=4) as NRM,
                tc.tile_pool(name="ysb", bufs=3) as YSB,
                tc.tile_pool(name="pss", bufs=2, space="PSUM") as PSS,
                tc.tile_pool(name="pso", bufs=1, space="PSUM") as PSO,
                tc.tile_pool(name="psm", bufs=2, space="PSUM") as PSM,
            ):
                for h in range(H):
                    qt, po = qk_t[h // 2], (h % 2) * 64
                    kt = qk_t[4 + h // 2]
                    p_tiles = [[], []]
                    for mi in range(8):
                        hu = HTU.tile([128, KH * N], u8, tag="hu", name="hu")
                        for k in range(KH):
                            nc.sync.dma_start(
                                out=hu[:, k * N:(k + 1) * N],
                                in_=hts_full[k * N + mi * 128: k * N + (mi + 1) * 128, :])
                        ht = HTP.tile([128, KH * N], bf16, tag="ht", name="ht")
                        nc.vector.tensor_copy(ht[:], hu[:])
                        for b in range(BPC):
                            t0 = b * N
                            ps = PSS.tile([128, N], f32, tag="pss", name="pss")
                            for nchunk in range(2):
                                sl = slice(nchunk * 512, (nchunk + 1) * 512)
                                nc.tensor.matmul(
                                    ps[:, sl],
                                    kt[po:po + 64, t0 + mi * 128: t0 + (mi + 1) * 128],
                                    qt[po:po + 64, t0 + nchunk * 512: t0 + (nchunk + 1) * 512],
                                    start=True, stop=False)
                                for k in range(KH):
                                    ci = (h * KH + k) * 128
                                    nc.tensor.matmul(
                                        ps[:, sl],
                                        ceye_t[:, ci:ci + 128],
                                        ht[:, k * N + nchunk * 512:
                                           k * N + (nchunk + 1) * 512],
                                        start=False, stop=(k == KH - 1))
                            pt = PP.tile([128, N], bf16, tag="p", name="p")
                            nc.scalar.activation(pt[:], ps[:], EXP)
                            p_tiles[b].append(pt)
                    for b in range(BPC):
                        pso = PSO.tile([HD + 1, N], f32, tag="pso", name="pso")
                        for mi in range(8):
                            for nchunk in range(2):
                                sl = slice(nchunk * 512, (nchunk + 1) * 512)
                                nc.tensor.matmul(
                                    pso[:, sl],
                                    v_t[b * 8 + mi][:, h * (HD + 1):(h + 1) * (HD + 1)],
                                    p_tiles[b][mi][:, sl],
                                    start=(mi == 0), stop=(mi == 7))
                        # denominator -> broadcast -> reciprocal -> normalize
                        d_t = NRM.tile([1, N], bf16, tag="d", name="d")
                        nc.vector.tensor_copy(d_t[:], pso[64:65, :])
                        R_t = NRM.tile([64, N], f32, tag="R", name="R")
                        for nchunk in range(2):
                            sl = slice(nchunk * 512, (nchunk + 1) * 512)
                            psr = PSM.tile([64, 512], f32, tag="psm", name="psm")
                            nc.tensor.matmul(psr[:], ones_t[:, 0:64], d_t[:, sl],
                                             start=True, stop=True)
                            nc.vector.reciprocal(R_t[:, sl], psr[:])
                        nc.vector.tensor_tensor(
                            oT_t[b * 4 + h // 2][po:po + 64, :],
                            pso[0:64, :], R_t[:], MUL)
                # ---------------- phase 3: output projection ----------------
                for b in range(BPC):
                    for t in range(8):
                        psy = PSM.tile([128, 512], f32, tag="psm", name="psm")
                        for c in range(4):
                            nc.tensor.matmul(
                                psy[:],
                                oT_t[b * 4 + c][:, t * 128:(t + 1) * 128],
                                wp_t[c][:], start=(c == 0), stop=(c == 3))
                        yt = YSB.tile([128, DIM], bf16, tag="y", name="y")
                        nc.vector.tensor_tensor(yt[:], psy[:], bpb_t[:], ADD)
                        nc.sync.dma_start(
                            out=y[b * N + t * 128: b * N + (t + 1) * 128, :],
                            in_=yt[:])
    nc.compile()
    return nc


def _prep_host(x, Hstack, hop_logits_attn, rel_alpha, Wqkv, Wproj, bproj):
    """Build the GLOBAL (concatenated-over-cores) input arrays directly.

    Shard layouts are chosen so that concatenating per-core shards along
    axis 0 reproduces the flat array itself - zero np.concatenate cost.
    Returns dict name -> global array of shape [8*per_core_rows, cols].
    """
    bf = ml_dtypes.bfloat16
    lg = hop_logits_attn - hop_logits_attn.max(-1, keepdims=True)
    w = np.exp(lg)
    w /= w.sum(-1, keepdims=True)                      # [H, KH]
    # Hstack ships as uint8 (values in [0,1], quantization err ~ bf16's);
    # the 1/255 dequant scale is folded into the scaled identities.
    c_hk = (rel_alpha[:, None] * w).astype(np.float32) / 255.0  # [H, KH]
    eye = np.eye(128, dtype=np.float32)
    ceye = (c_hk.reshape(H * KH, 1, 1) * eye).astype(bf).reshape(CER, 128)
    hts = (Hstack.transpose(0, 2, 1) * 255.0 + 0.5).astype(np.uint8).reshape(HTR, N)
    wqkvT = np.ascontiguousarray(Wqkv.T).astype(np.float32)
    wqkvT[:, :DIM] *= SCALE                            # fold q scaling
    wqkvT = wqkvT.astype(bf)
    wprojT = np.ascontiguousarray(Wproj.T).astype(bf)
    return {
        "xn": x.reshape(NCORES * TOK, DIM).astype(bf),
        "hts_in": hts,
        "wqkv_in": wqkvT,
        "wproj_in": wprojT,
        "ceye_in": ceye,
        "eye_in": eye.astype(bf),
        "bprojr": np.tile(bproj.astype(np.float32)[None, :], (NCORES, 1)),
    }


def _make_runner(nc):
    """Persistent-jit runner for the bass_exec custom call (the axon/PJRT
    path), so warm calls skip tracing and output zero-buffers are created
    on device instead of being shipped from the host."""
    import jax
    import jax.numpy as jnp
    from jax.sharding import Mesh, PartitionSpec, NamedSharding
    from jax.experimental.shard_map import shard_map
    from concourse import mybir
    from concourse.bass2jax import (
        _bass_exec_p, partition_id_tensor, install_neuronx_cc_hook)

    install_neuronx_cc_hook()
    partition_name = nc.partition_id_tensor.name if nc.partition_id_tensor else None
    in_names, out_names, out_avals = [], [], []
    for alloc in nc.m.functions[0].allocations:
        if not isinstance(alloc, mybir.MemoryLocationSet):
            continue
        name = alloc.memorylocations[0].name
        if alloc.kind == "ExternalInput":
            if name != partition_name:
                in_names.append(name)
        elif alloc.kind == "ExternalOutput":
            out_names.append(name)
            out_avals.append(jax.core.ShapedArray(
                tuple(alloc.tensor_shape), mybir.dt.np(alloc.dtype)))
    n_params = len(in_names)
    n_outs = len(out_avals)
    all_names = in_names + out_names
    if partition_name is not None:
        all_names = all_names + [partition_name]
    donate = tuple(range(n_params, n_params + n_outs))

    def _body(*args):
        operands = list(args)
        if partition_name is not None:
            operands.append(partition_id_tensor())
        outs = _bass_exec_p.bind(
            *operands, out_avals=tuple(out_avals), in_names=tuple(all_names),
            out_names=tuple(out_names), lowering_input_output_aliases=(),
            sim_require_finite=True, sim_require_nnan=True, nc=nc)
        return tuple(outs)

    devices = jax.devices()[:NCORES]
    mesh = Mesh(np.asarray(devices), ("core",))
    spec = NamedSharding(mesh, PartitionSpec("core"))
    in_specs = (PartitionSpec("core"),) * (n_params + n_outs)
    out_specs = (PartitionSpec("core"),) * n_outs
    sharded = jax.jit(
        shard_map(_body, mesh=mesh, in_specs=in_specs, out_specs=out_specs,
                  check_rep=False),
        donate_argnums=donate, keep_unused=True)

    zero_shapes = [(NCORES * a.shape[0], *a.shape[1:]) for a in out_avals]
    zero_dtypes = [a.dtype for a in out_avals]
    zeros_fn = jax.jit(
        lambda: tuple(jnp.zeros(s, d) for s, d in zip(zero_shapes, zero_dtypes)),
        out_shardings=tuple(spec for _ in out_avals))

    prev = []

    def run(global_in: dict):
        ins = [global_in[name] for name in in_names]
        # donate the previous call's (already fetched) output buffers as the
        # custom call's result allocation; first call builds zeros on device
        zs = tuple(prev) if prev else zeros_fn()
        prev.clear()
        outs = sharded(*ins, *zs)
        res = {name: np.asarray(o) for name, o in zip(out_names, outs)}
        prev.extend(outs)
        return res

    return run


def kernel(**inputs):
    if "run" not in _CACHE:
        _CACHE["nc"] = _build()
        _CACHE["run"] = _make_runner(_CACHE["nc"])
    gin = _prep_host(
        np.asarray(inputs["x"], np.float32),
        np.asarray(inputs["Hstack"], np.float32),
        np.asarray(inputs["hop_logits_attn"], np.float32),
        np.asarray(inputs["rel_alpha"], np.float32),
        np.asarray(inputs["Wqkv"], np.float32),
        np.asarray(inputs["Wproj"], np.float32),
        np.asarray(inputs["bproj"], np.float32))
    outs = _CACHE["run"](gin)
    return outs["y"].astype(np.float32).reshape(B, N, DIM)


# revision 24
# speedup vs baseline: 4.7532x; 1.0236x over previous
"""Trainium2 Bass kernel for nn_Attention_xxc (dense transformer attention
with hop-distance bias). Data-parallel over batch: 8 cores x 2 batches.

Wire-traffic-minimized design: the warm end-to-end latency of this problem
is dominated by host<->device transfer over the axon tunnel (~50 MB/s), so
every shared tensor is shipped sharded 1/8-per-core and AllGathered on
device over NeuronLink; the hop-bias mixture  alpha_h * sum_k w_hk Hstack_k
is never materialized on the host - the PE folds it into the score matmuls
as  S.T = K^T Q + sum_k (c_hk I) @ Hstack_k.T  accumulated in PSUM.

Per-core layout (core c of 8):
  - xn [2048, 512] bf16: the core's own 2 batches, natural layout; the PE
    transposes it on device via identity matmuls.
  - shards (rows c/8) of: HTs flat [5120,1024] (Hstack_k transposed),
    wqkvT [512,1536] (q cols pre-scaled 1/sqrt(hd)), wprojT [512,512],
    ceye flat [5120,128] (40 scaled identities c_hk*I), eye128.
  - qkv: q,k TRANSPOSED ([outch, tok] bf16), v NATURAL with a ones column
    per head (65 cols/head) so the AV matmul also produces the softmax
    denominator in row 64.
  - output y [2048, 512] bf16, host casts to f32.
Runner: persistent jax jit of the bass_exec custom call (no per-call
retrace), donated output buffers are created on device (no host zeros).
"""
import sys

sys.path.insert(0, "/opt/trn_rl_repo")

import numpy as np
import ml_dtypes

B, N, DIM = 16, 1024, 512
H, HD, KH = 8, 64, 5
SCALE = HD ** -0.5
NCORES = 8
BPC = B // NCORES          # batches per core
TOK = BPC * N              # tokens per core = 2048
HTR = KH * N               # 5120 rows of flat transposed-Hstack
CER = H * KH * 128         # 5120 rows of flat scaled-identity stack

# shared-blob layout, in rows of 1024 bytes (= 512 bf16 / 1024 u8):
#   wqkv bf16 [512,1536] | wproj bf16 [512,512] | ceye bf16 [5120,128]
#   | eye bf16 [128,128] | bproj bf16 [512] | hts u8 [5120,1024] | pad
R_WQKV = 0
R_WPROJ = R_WQKV + 512 * 3
R_CEYE = R_WPROJ + 512
R_EYE = R_CEYE + CER // 4
R_BPROJ = R_EYE + 32
R_HTS = R_BPROJ + 1
SHR_ROWS = -(-(R_HTS + HTR) // 8) * 8    # pad to a multiple of 8 cores

_CACHE = {}


def _build():
    import concourse.bass as bass
    import concourse.bacc as bacc
    import concourse.mybir as mybir
    from concourse.tile import TileContext

    f32 = mybir.dt.float32
    bf16 = mybir.dt.bfloat16
    u8 = mybir.dt.uint8
    EXP = mybir.ActivationFunctionType.Exp
    MUL = mybir.AluOpType.mult
    ADD = mybir.AluOpType.add
    BYP = mybir.AluOpType.bypass
    RG = [list(range(NCORES))]

    nc = bacc.Bacc(num_devices=NCORES)
    xn = nc.declare_dram_parameter("xn", [TOK, DIM], bf16, isOutput=False)
    shr_in = nc.declare_dram_parameter("shr_in", [SHR_ROWS // 8, 1024], u8, isOutput=False)
    y = nc.declare_dram_parameter("y", [TOK, DIM], bf16, isOutput=True)

    NT = TOK // 128            # 16 token tiles per core
    VW = H * (HD + 1)          # 520: v row width with ones col per head

    with TileContext(nc) as tc:
        with (
            tc.tile_pool(name="dram", bufs=1, space="DRAM") as DR,
            tc.tile_pool(name="qk", bufs=1) as QK,
            tc.tile_pool(name="vres", bufs=1) as VR,
            tc.tile_pool(name="wp", bufs=1) as WP,
            tc.tile_pool(name="outT", bufs=1) as OT,
            tc.tile_pool(name="const", bufs=1) as CONST,
        ):
            # ---------------- phase 0: AllGather the one shared blob ----------------
            bnc = DR.tile([SHR_ROWS // 8, 1024], u8, tag="b_shr", name="b_shr")
            shr_full = DR.tile([SHR_ROWS, 1024], u8, tag="g_shr", name="g_shr")
            nc.gpsimd.dma_start(bnc[:], shr_in[:])
            nc.gpsimd.collective_compute(
                "AllGather", BYP, replica_groups=RG,
                ins=[bnc.opt()], outs=[shr_full.opt()])

            eye_t = CONST.tile([128, 128], bf16, tag="eye", name="eye")
            nc.sync.dma_start(
                out=eye_t[:],
                in_=shr_full[R_EYE: R_EYE + 32, :].bitcast(bf16)
                .rearrange("a (b c) -> (a b) c", b=4))
            ones_t = CONST.tile([1, 128], bf16, tag="ones", name="ones")
            nc.vector.memset(ones_t[:], 1.0)
            ceye_t = CONST.tile([128, H * KH * 128], bf16, tag="ceye", name="ceye")
            for j in range(H * KH):
                nc.sync.dma_start(
                    out=ceye_t[:, j * 128:(j + 1) * 128],
                    in_=shr_full[R_CEYE + 32 * j: R_CEYE + 32 * (j + 1), :]
                    .bitcast(bf16).rearrange("a (b c) -> (a b) c", b=4))
            wp_t = [WP.tile([128, DIM], bf16, tag=f"wp{c}", name=f"wp{c}") for c in range(4)]
            for c in range(4):
                nc.sync.dma_start(
                    out=wp_t[c][:],
                    in_=shr_full[R_WPROJ + c * 128: R_WPROJ + (c + 1) * 128, :]
                    .bitcast(bf16))

            qk_t = [QK.tile([128, TOK], bf16, tag=f"qk{o}", name=f"qk{o}") for o in range(8)]
            v_t = [VR.tile([128, VW], bf16, tag=f"v{t}", name=f"v{t}") for t in range(NT)]
            oT_t = [OT.tile([128, N], bf16, tag=f"oT{b}_{c}", name=f"oT{b}_{c}")
                    for b in range(BPC) for c in range(4)]

            # broadcast bproj across 128 partitions: ones^T [128] x bproj [1,512]
            bpb_t = CONST.tile([128, DIM], f32, tag="bpb", name="bpb")
            bpr_t = CONST.tile([1, DIM], bf16, tag="bpr", name="bpr")
            nc.sync.dma_start(out=bpr_t[:],
                              in_=shr_full[R_BPROJ: R_BPROJ + 1, :].bitcast(bf16))

            # ---------------- phase 1: x transpose + qkv projections ----------------
            with (
                tc.tile_pool(name="xw", bufs=1) as XW,
                tc.tile_pool(name="ps1", bufs=4, space="PSUM") as PS1,
                tc.tile_pool(name="pst", bufs=4, space="PSUM") as PST,
            ):
                psb = PS1.tile([128, DIM], f32, tag="ps1", name="ps1")
                nc.tensor.matmul(psb[:], ones_t[:], bpr_t[:], start=True, stop=True)
                nc.vector.tensor_copy(bpb_t[:], psb[:])

                xn_t = [XW.tile([128, DIM], bf16, tag=f"xn{t}", name=f"xn{t}")
                        for t in range(NT)]
                for t in range(NT):
                    nc.sync.dma_start(out=xn_t[t][:], in_=xn[t * 128:(t + 1) * 128, :])
                xT_t = [XW.tile([128, TOK], bf16, tag=f"x{c}", name=f"x{c}") for c in range(4)]
                for t in range(NT):
                    for c in range(4):
                        pst = PST.tile([128, 128], f32, tag="pst", name="pst")
                        nc.tensor.matmul(pst[:], xn_t[t][:, c * 128:(c + 1) * 128],
                                         eye_t[:], start=True, stop=True)
                        nc.vector.tensor_copy(xT_t[c][:, t * 128:(t + 1) * 128], pst[:])

                wq_t = [XW.tile([128, 3 * DIM], bf16, tag=f"w{c}", name=f"w{c}") for c in range(4)]
                for c in range(4):
                    for t in range(3):
                        nc.sync.dma_start(
                            out=wq_t[c][:, 512 * t:512 * (t + 1)],
                            in_=shr_full[R_WQKV + 384 * c + t:
                                         R_WQKV + 384 * (c + 1): 3, :].bitcast(bf16))

                # q,k transposed: qkvT[o_tile, tok] ; o tiles 0..7 cover q,k
                for o in range(8):
                    for t in range(4):           # tok chunks of 512
                        ps = PS1.tile([128, 512], f32, tag="ps1", name="ps1")
                        for c in range(4):
                            nc.tensor.matmul(
                                ps[:], wq_t[c][:, o * 128:(o + 1) * 128],
                                xT_t[c][:, t * 512:(t + 1) * 512],
                                start=(c == 0), stop=(c == 3))
                        nc.vector.tensor_copy(qk_t[o][:, t * 512:(t + 1) * 512], ps[:])
                # v natural: [tok_tile, vch] -> packed per head with ones col
                for t in range(NT):
                    ps = PS1.tile([128, 512], f32, tag="ps1", name="ps1")
                    for c in range(4):
                        nc.tensor.matmul(
                            ps[:], xT_t[c][:, t * 128:(t + 1) * 128],
                            wq_t[c][:, 2 * DIM:3 * DIM],
                            start=(c == 0), stop=(c == 3))
                    dst = v_t[t][:, 0:VW].rearrange("p (h s) -> p h s", s=HD + 1)
                    nc.vector.tensor_copy(
                        dst[:, :, 0:HD],
                        ps[:].rearrange("p (h s) -> p h s", s=HD))
                    nc.vector.memset(dst[:, :, HD:HD + 1], 1.0)

            # ---------------- phase 2: attention ----------------
            with (
                tc.tile_pool(name="htu", bufs=3) as HTU,
                tc.tile_pool(name="htp", bufs=4) as HTP,
                tc.tile_pool(name="pp", bufs=18) as PP,
                tc.tile_pool(name="nrm", bufs=4) as NRM,
                tc.tile_pool(name="ysb", bufs=3) as YSB,
                tc.tile_pool(name="pss", bufs=2, space="PSUM") as PSS,
                tc.tile_pool(name="pso", bufs=1, space="PSUM") as PSO,
                tc.tile_pool(name="psm", bufs=2, space="PSUM") as PSM,
            ):
                for h in range(H):
                    qt, po = qk_t[h // 2], (h % 2) * 64
                    kt = qk_t[4 + h // 2]
                    p_tiles = [[], []]
                    for mi in range(8):
                        hu = HTU.tile([128, KH * N], u8, tag="hu", name="hu")
                        for k in range(KH):
                            nc.sync.dma_start(
                                out=hu[:, k * N:(k + 1) * N],
                                in_=shr_full[R_HTS + k * N + mi * 128:
                                             R_HTS + k * N + (mi + 1) * 128, :])
                        ht = HTP.tile([128, KH * N], bf16, tag="ht", name="ht")
                        nc.vector.tensor_copy(ht[:], hu[:])
                        for b in range(BPC):
                            t0 = b * N
                            ps = PSS.tile([128, N], f32, tag="pss", name="pss")
                            for nchunk in range(2):
                                sl = slice(nchunk * 512, (nchunk + 1) * 512)
                                nc.tensor.matmul(
                                    ps[:, sl],
                                    kt[po:po + 64, t0 + mi * 128: t0 + (mi + 1) * 128],
                                    qt[po:po + 64, t0 + nchunk * 512: t0 + (nchunk + 1) * 512],
                                    start=True, stop=False)
                                for k in range(KH):
                                    ci = (h * KH + k) * 128
                                    nc.tensor.matmul(
                                        ps[:, sl],
                                        ceye_t[:, ci:ci + 128],
                                        ht[:, k * N + nchunk * 512:
                                           k * N + (nchunk + 1) * 512],
                                        start=False, stop=(k == KH - 1))
                            pt = PP.tile([128, N], bf16, tag="p", name="p")
                            nc.scalar.activation(pt[:], ps[:], EXP)
                            p_tiles[b].append(pt)
                    for b in range(BPC):
                        pso = PSO.tile([HD + 1, N], f32, tag="pso", name="pso")
                        for mi in range(8):
                            for nchunk in range(2):
                                sl = slice(nchunk * 512, (nchunk + 1) * 512)
                                nc.tensor.matmul(
                                    pso[:, sl],
                                    v_t[b * 8 + mi][:, h * (HD + 1):(h + 1) * (HD + 1)],
                                    p_tiles[b][mi][:, sl],
                                    start=(mi == 0), stop=(mi == 7))
                        # denominator -> broadcast -> reciprocal -> normalize
                        d_t = NRM.tile([1, N], bf16, tag="d", name="d")
                        nc.vector.tensor_copy(d_t[:], pso[64:65, :])
                        R_t = NRM.tile([64, N], f32, tag="R", name="R")
                        for nchunk in range(2):
                            sl = slice(nchunk * 512, (nchunk + 1) * 512)
                            psr = PSM.tile([64, 512], f32, tag="psm", name="psm")
                            nc.tensor.matmul(psr[:], ones_t[:, 0:64], d_t[:, sl],
                                             start=True, stop=True)
                            nc.vector.reciprocal(R_t[:, sl], psr[:])
                        nc.vector.tensor_tensor(
                            oT_t[b * 4 + h // 2][po:po + 64, :],
                            pso[0:64, :], R_t[:], MUL)
                # ---------------- phase 3: output projection ----------------
                for b in range(BPC):
                    for t in range(8):
                        psy = PSM.tile([128, 512], f32, tag="psm", name="psm")
                        for c in range(4):
                            nc.tensor.matmul(
                                psy[:],
                                oT_t[b * 4 + c][:, t * 128:(t + 1) * 128],
                                wp_t[c][:], start=(c == 0), stop=(c == 3))
                        yt = YSB.tile([128, DIM], bf16, tag="y", name="y")
                        nc.vector.tensor_tensor(yt[:], psy[:], bpb_t[:], ADD)
                        nc.sync.dma_start(
                            out=y[b * N + t * 128: b * N + (t + 1) * 128, :],
                            in_=yt[:])
    nc.compile()
    return nc


def _prep_host(x, Hstack, hop_logits_attn, rel_alpha, Wqkv, Wproj, bproj):
    """Build the GLOBAL (concatenated-over-cores) input arrays directly.

    Shard layouts are chosen so that concatenating per-core shards along
    axis 0 reproduces the flat array itself - zero np.concatenate cost.
    Returns dict name -> global array of shape [8*per_core_rows, cols].
    """
    bf = ml_dtypes.bfloat16
    lg = hop_logits_attn - hop_logits_attn.max(-1, keepdims=True)
    w = np.exp(lg)
    w /= w.sum(-1, keepdims=True)                      # [H, KH]
    # Hstack ships as uint8 (values in [0,1], quantization err ~ bf16's);
    # the 1/255 dequant scale is folded into the scaled identities.
    c_hk = (rel_alpha[:, None] * w).astype(np.float32) / 255.0  # [H, KH]
    eye = np.eye(128, dtype=np.float32)
    ceye = (c_hk.reshape(H * KH, 1, 1) * eye).astype(bf).reshape(CER, 128)
    hts = (Hstack.transpose(0, 2, 1) * 255.0 + 0.5).astype(np.uint8).reshape(HTR, N)
    wqkvT = np.ascontiguousarray(Wqkv.T).astype(np.float32)
    wqkvT[:, :DIM] *= SCALE                            # fold q scaling
    wqkvT = wqkvT.astype(bf)
    wprojT = np.ascontiguousarray(Wproj.T).astype(bf)
    u8row = lambda a: np.ascontiguousarray(a).view(np.uint8).reshape(-1, 1024)
    shr = np.concatenate([
        u8row(wqkvT), u8row(wprojT), u8row(ceye), u8row(eye.astype(bf)),
        u8row(bproj.astype(bf)[None, :]), hts,
        np.zeros((SHR_ROWS - R_HTS - HTR, 1024), np.uint8),
    ], axis=0)
    return {
        "xn": x.reshape(NCORES * TOK, DIM).astype(bf),
        "shr_in": shr,
    }


def _make_runner(nc):
    """Persistent-jit runner for the bass_exec custom call (the axon/PJRT
    path), so warm calls skip tracing and output zero-buffers are created
    on device instead of being shipped from the host."""
    import jax
    import jax.numpy as jnp
    from jax.sharding import Mesh, PartitionSpec, NamedSharding
    from jax.experimental.shard_map import shard_map
    from concourse import mybir
    from concourse.bass2jax import (
        _bass_exec_p, partition_id_tensor, install_neuronx_cc_hook)

    install_neuronx_cc_hook()
    partition_name = nc.partition_id_tensor.name if nc.partition_id_tensor else None
    in_names, out_names, out_avals = [], [], []
    for alloc in nc.m.functions[0].allocations:
        if not isinstance(alloc, mybir.MemoryLocationSet):
            continue
        name = alloc.memorylocations[0].name
        if alloc.kind == "ExternalInput":
            if name != partition_name:
                in_names.append(name)
        elif alloc.kind == "ExternalOutput":
            out_names.append(name)
            out_avals.append(jax.core.ShapedArray(
                tuple(alloc.tensor_shape), mybir.dt.np(alloc.dtype)))
    n_params = len(in_names)
    n_outs = len(out_avals)
    all_names = in_names + out_names
    if partition_name is not None:
        all_names = all_names + [partition_name]
    donate = tuple(range(n_params, n_params + n_outs))

    def _body(*args):
        operands = list(args)
        if partition_name is not None:
            operands.append(partition_id_tensor())
        outs = _bass_exec_p.bind(
            *operands, out_avals=tuple(out_avals), in_names=tuple(all_names),
            out_names=tuple(out_names), lowering_input_output_aliases=(),
            sim_require_finite=True, sim_require_nnan=True, nc=nc)
        return tuple(outs)

    devices = jax.devices()[:NCORES]
    mesh = Mesh(np.asarray(devices), ("core",))
    spec = NamedSharding(mesh, PartitionSpec("core"))
    in_specs = (PartitionSpec("core"),) * (n_params + n_outs)
    out_specs = (PartitionSpec("core"),) * n_outs
    sharded = jax.jit(
        shard_map(_body, mesh=mesh, in_specs=in_specs, out_specs=out_specs,
                  check_rep=False),
        donate_argnums=donate, keep_unused=True)

    zero_shapes = [(NCORES * a.shape[0], *a.shape[1:]) for a in out_avals]
    zero_dtypes = [a.dtype for a in out_avals]
    zeros_fn = jax.jit(
        lambda: tuple(jnp.zeros(s, d) for s, d in zip(zero_shapes, zero_dtypes)),
        out_shardings=tuple(spec for _ in out_avals))

    prev = []

    def run(global_in: dict):
        ins = [global_in[name] for name in in_names]
        # donate the previous call's (already fetched) output buffers as the
        # custom call's result allocation; first call builds zeros on device
        zs = tuple(prev) if prev else zeros_fn()
        prev.clear()
        outs = sharded(*ins, *zs)
        res = {name: np.asarray(o) for name, o in zip(out_names, outs)}
        prev.extend(outs)
        return res

    return run


def kernel(**inputs):
    if "run" not in _CACHE:
        _CACHE["nc"] = _build()
        _CACHE["run"] = _make_runner(_CACHE["nc"])
    gin = _prep_host(
        np.asarray(inputs["x"], np.float32),
        np.asarray(inputs["Hstack"], np.float32),
        np.asarray(inputs["hop_logits_attn"], np.float32),
        np.asarray(inputs["rel_alpha"], np.float32),
        np.asarray(inputs["Wqkv"], np.float32),
        np.asarray(inputs["Wproj"], np.float32),
        np.asarray(inputs["bproj"], np.float32))
    outs = _CACHE["run"](gin)
    return outs["y"].astype(np.float32).reshape(B, N, DIM)


# revision 28
# speedup vs baseline: 5.3813x; 1.1322x over previous
"""Trainium2 Bass kernel for nn_Attention_xxc (dense transformer attention
with hop-distance bias). Data-parallel over batch: 8 cores x 2 batches.

Wire-traffic-minimized design: the warm end-to-end latency of this problem
is dominated by host<->device transfer over the axon tunnel (~50 MB/s), so
every shared tensor is shipped sharded 1/8-per-core and AllGathered on
device over NeuronLink; the hop-bias mixture  alpha_h * sum_k w_hk Hstack_k
is never materialized on the host - the PE folds it into the score matmuls
as  S.T = K^T Q + sum_k (c_hk I) @ Hstack_k.T  accumulated in PSUM.

Per-core layout (core c of 8):
  - xn [2048, 512] bf16: the core's own 2 batches, natural layout; the PE
    transposes it on device via identity matmuls.
  - shards (rows c/8) of: HTs flat [5120,1024] (Hstack_k transposed),
    wqkvT [512,1536] (q cols pre-scaled 1/sqrt(hd)), wprojT [512,512],
    ceye flat [5120,128] (40 scaled identities c_hk*I), eye128.
  - qkv: q,k TRANSPOSED ([outch, tok] bf16), v NATURAL with a ones column
    per head (65 cols/head) so the AV matmul also produces the softmax
    denominator in row 64.
  - output y [2048, 512] bf16, host casts to f32.
Runner: persistent jax jit of the bass_exec custom call (no per-call
retrace), donated output buffers are created on device (no host zeros).
"""
import sys

sys.path.insert(0, "/opt/trn_rl_repo")

import numpy as np
import ml_dtypes

B, N, DIM = 16, 1024, 512
H, HD, KH = 8, 64, 5
SCALE = HD ** -0.5
NCORES = 8
BPC = B // NCORES          # batches per core
TOK = BPC * N              # tokens per core = 2048
HTR = KH * N               # 5120 rows of flat transposed-Hstack
CER = H * KH * 128         # 5120 rows of flat scaled-identity stack

# shared-blob layout, in rows of 1024 bytes (= 512 bf16 / 1024 u8):
#   wqkv bf16 [512,1536] | wproj bf16 [512,512] | ceye bf16 [5120,128]
#   | eye bf16 [128,128] | bproj bf16 [512] | hts u8 [5120,1024] | pad
R_WQKV = 0
R_WPROJ = R_WQKV + 512 * 3
R_CEYE = R_WPROJ + 512
R_EYE = R_CEYE + CER // 4
R_BPROJ = R_EYE + 32
R_HTS = R_BPROJ + 1
SHR_ROWS = -(-(R_HTS + HTR) // 8) * 8    # pad to a multiple of 8 cores

_CACHE = {}


def _build():
    import concourse.bass as bass
    import concourse.bacc as bacc
    import concourse.mybir as mybir
    from concourse.tile import TileContext

    f32 = mybir.dt.float32
    bf16 = mybir.dt.bfloat16
    u8 = mybir.dt.uint8
    f8 = mybir.dt.float8e4
    EXP = mybir.ActivationFunctionType.Exp
    MUL = mybir.AluOpType.mult
    ADD = mybir.AluOpType.add
    BYP = mybir.AluOpType.bypass
    RG = [list(range(NCORES))]

    nc = bacc.Bacc(num_devices=NCORES)
    xn = nc.declare_dram_parameter("xn", [TOK, DIM], f8, isOutput=False)
    shr_in = nc.declare_dram_parameter("shr_in", [SHR_ROWS // 8, 1024], u8, isOutput=False)
    y = nc.declare_dram_parameter("y", [TOK, DIM], bf16, isOutput=True)

    NT = TOK // 128            # 16 token tiles per core
    VW = H * (HD + 1)          # 520: v row width with ones col per head

    with TileContext(nc) as tc:
        with (
            tc.tile_pool(name="dram", bufs=1, space="DRAM") as DR,
            tc.tile_pool(name="qk", bufs=1) as QK,
            tc.tile_pool(name="vres", bufs=1) as VR,
            tc.tile_pool(name="wp", bufs=1) as WP,
            tc.tile_pool(name="outT", bufs=1) as OT,
            tc.tile_pool(name="const", bufs=1) as CONST,
        ):
            # ---------------- phase 0: AllGather the one shared blob ----------------
            bnc = DR.tile([SHR_ROWS // 8, 1024], u8, tag="b_shr", name="b_shr")
            shr_full = DR.tile([SHR_ROWS, 1024], u8, tag="g_shr", name="g_shr")
            nc.gpsimd.dma_start(bnc[:], shr_in[:])
            nc.gpsimd.collective_compute(
                "AllGather", BYP, replica_groups=RG,
                ins=[bnc.opt()], outs=[shr_full.opt()])

            eye_t = CONST.tile([128, 128], bf16, tag="eye", name="eye")
            nc.sync.dma_start(
                out=eye_t[:],
                in_=shr_full[R_EYE: R_EYE + 32, :].bitcast(bf16)
                .rearrange("a (b c) -> (a b) c", b=4))
            ones_t = CONST.tile([1, 128], bf16, tag="ones", name="ones")
            nc.vector.memset(ones_t[:], 1.0)
            ceye_t = CONST.tile([128, H * KH * 128], bf16, tag="ceye", name="ceye")
            for j in range(H * KH):
                nc.sync.dma_start(
                    out=ceye_t[:, j * 128:(j + 1) * 128],
                    in_=shr_full[R_CEYE + 32 * j: R_CEYE + 32 * (j + 1), :]
                    .bitcast(bf16).rearrange("a (b c) -> (a b) c", b=4))
            wp_t = [WP.tile([128, DIM], bf16, tag=f"wp{c}", name=f"wp{c}") for c in range(4)]
            for c in range(4):
                nc.sync.dma_start(
                    out=wp_t[c][:],
                    in_=shr_full[R_WPROJ + c * 128: R_WPROJ + (c + 1) * 128, :]
                    .bitcast(bf16))

            qk_t = [QK.tile([128, TOK], bf16, tag=f"qk{o}", name=f"qk{o}") for o in range(8)]
            v_t = [VR.tile([128, VW], bf16, tag=f"v{t}", name=f"v{t}") for t in range(NT)]
            oT_t = [OT.tile([128, N], bf16, tag=f"oT{b}_{c}", name=f"oT{b}_{c}")
                    for b in range(BPC) for c in range(4)]

            # broadcast bproj across 128 partitions: ones^T [128] x bproj [1,512]
            bpb_t = CONST.tile([128, DIM], f32, tag="bpb", name="bpb")
            bpr_t = CONST.tile([1, DIM], bf16, tag="bpr", name="bpr")
            nc.sync.dma_start(out=bpr_t[:],
                              in_=shr_full[R_BPROJ: R_BPROJ + 1, :].bitcast(bf16))

            # ---------------- phase 1: x transpose + qkv projections ----------------
            with (
                tc.tile_pool(name="xw", bufs=1) as XW,
                tc.tile_pool(name="ps1", bufs=4, space="PSUM") as PS1,
                tc.tile_pool(name="pst", bufs=4, space="PSUM") as PST,
            ):
                psb = PS1.tile([128, DIM], f32, tag="ps1", name="ps1")
                nc.tensor.matmul(psb[:], ones_t[:], bpr_t[:], start=True, stop=True)
                nc.vector.tensor_copy(bpb_t[:], psb[:])

                xn_t = [XW.tile([128, DIM], bf16, tag=f"xn{t}", name=f"xn{t}")
                        for t in range(NT)]
                for t in range(NT):
                    x8 = XW.tile([128, DIM], f8, tag=f"x8_{t}", name=f"x8_{t}")
                    nc.sync.dma_start(out=x8[:], in_=xn[t * 128:(t + 1) * 128, :])
                    nc.vector.tensor_copy(xn_t[t][:], x8[:])
                xT_t = [XW.tile([128, TOK], bf16, tag=f"x{c}", name=f"x{c}") for c in range(4)]
                for t in range(NT):
                    for c in range(4):
                        pst = PST.tile([128, 128], f32, tag="pst", name="pst")
                        nc.tensor.matmul(pst[:], xn_t[t][:, c * 128:(c + 1) * 128],
                                         eye_t[:], start=True, stop=True)
                        nc.vector.tensor_copy(xT_t[c][:, t * 128:(t + 1) * 128], pst[:])

                wq_t = [XW.tile([128, 3 * DIM], bf16, tag=f"w{c}", name=f"w{c}") for c in range(4)]
                for c in range(4):
                    for t in range(3):
                        nc.sync.dma_start(
                            out=wq_t[c][:, 512 * t:512 * (t + 1)],
                            in_=shr_full[R_WQKV + 384 * c + t:
                                         R_WQKV + 384 * (c + 1): 3, :].bitcast(bf16))

                # q,k transposed: qkvT[o_tile, tok] ; o tiles 0..7 cover q,k
                for o in range(8):
                    for t in range(4):           # tok chunks of 512
                        ps = PS1.tile([128, 512], f32, tag="ps1", name="ps1")
                        for c in range(4):
                            nc.tensor.matmul(
                                ps[:], wq_t[c][:, o * 128:(o + 1) * 128],
                                xT_t[c][:, t * 512:(t + 1) * 512],
                                start=(c == 0), stop=(c == 3))
                        nc.vector.tensor_copy(qk_t[o][:, t * 512:(t + 1) * 512], ps[:])
                # v natural: [tok_tile, vch] -> packed per head with ones col
                for t in range(NT):
                    ps = PS1.tile([128, 512], f32, tag="ps1", name="ps1")
                    for c in range(4):
                        nc.tensor.matmul(
                            ps[:], xT_t[c][:, t * 128:(t + 1) * 128],
                            wq_t[c][:, 2 * DIM:3 * DIM],
                            start=(c == 0), stop=(c == 3))
                    dst = v_t[t][:, 0:VW].rearrange("p (h s) -> p h s", s=HD + 1)
                    nc.vector.tensor_copy(
                        dst[:, :, 0:HD],
                        ps[:].rearrange("p (h s) -> p h s", s=HD))
                    nc.vector.memset(dst[:, :, HD:HD + 1], 1.0)

            # ---------------- phase 2: attention ----------------
            with (
                tc.tile_pool(name="htu", bufs=3) as HTU,
                tc.tile_pool(name="htp", bufs=4) as HTP,
                tc.tile_pool(name="pp", bufs=18) as PP,
                tc.tile_pool(name="nrm", bufs=4) as NRM,
                tc.tile_pool(name="ysb", bufs=3) as YSB,
                tc.tile_pool(name="pss", bufs=2, space="PSUM") as PSS,
                tc.tile_pool(name="pso", bufs=1, space="PSUM") as PSO,
                tc.tile_pool(name="psm", bufs=2, space="PSUM") as PSM,
            ):
                for h in range(H):
                    qt, po = qk_t[h // 2], (h % 2) * 64
                    kt = qk_t[4 + h // 2]
                    p_tiles = [[], []]
                    for mi in range(8):
                        hu = HTU.tile([128, KH * N], u8, tag="hu", name="hu")
                        for k in range(KH):
                            nc.sync.dma_start(
                                out=hu[:, k * N:(k + 1) * N],
                                in_=shr_full[R_HTS + k * N + mi * 128:
                                             R_HTS + k * N + (mi + 1) * 128, :])
                        ht = HTP.tile([128, KH * N], bf16, tag="ht", name="ht")
                        nc.vector.tensor_copy(ht[:], hu[:])
                        for b in range(BPC):
                            t0 = b * N
                            ps = PSS.tile([128, N], f32, tag="pss", name="pss")
                            for nchunk in range(2):
                                sl = slice(nchunk * 512, (nchunk + 1) * 512)
                                nc.tensor.matmul(
                                    ps[:, sl],
                                    kt[po:po + 64, t0 + mi * 128: t0 + (mi + 1) * 128],
                                    qt[po:po + 64, t0 + nchunk * 512: t0 + (nchunk + 1) * 512],
                                    start=True, stop=False)
                                for k in range(KH):
                                    ci = (h * KH + k) * 128
                                    nc.tensor.matmul(
                                        ps[:, sl],
                                        ceye_t[:, ci:ci + 128],
                                        ht[:, k * N + nchunk * 512:
                                           k * N + (nchunk + 1) * 512],
                                        start=False, stop=(k == KH - 1))
                            pt = PP.tile([128, N], bf16, tag="p", name="p")
                            nc.scalar.activation(pt[:], ps[:], EXP)
                            p_tiles[b].append(pt)
                    for b in range(BPC):
                        pso = PSO.tile([HD + 1, N], f32, tag="pso", name="pso")
                        for mi in range(8):
                            for nchunk in range(2):
                                sl = slice(nchunk * 512, (nchunk + 1) * 512)
                                nc.tensor.matmul(
                                    pso[:, sl],
                                    v_t[b * 8 + mi][:, h * (HD + 1):(h + 1) * (HD + 1)],
                                    p_tiles[b][mi][:, sl],
                                    start=(mi == 0), stop=(mi == 7))
                        # denominator -> broadcast -> reciprocal -> normalize
                        d_t = NRM.tile([1, N], bf16, tag="d", name="d")
                        nc.vector.tensor_copy(d_t[:], pso[64:65, :])
                        R_t = NRM.tile([64, N], f32, tag="R", name="R")
                        for nchunk in range(2):
                            sl = slice(nchunk * 512, (nchunk + 1) * 512)
                            psr = PSM.tile([64, 512], f32, tag="psm", name="psm")
                            nc.tensor.matmul(psr[:], ones_t[:, 0:64], d_t[:, sl],
                                             start=True, stop=True)
                            nc.vector.reciprocal(R_t[:, sl], psr[:])
                        nc.vector.tensor_tensor(
                            oT_t[b * 4 + h // 2][po:po + 64, :],
                            pso[0:64, :], R_t[:], MUL)
                # ---------------- phase 3: output projection ----------------
                for b in range(BPC):
                    for t in range(8):
                        psy = PSM.tile([128, 512], f32, tag="psm", name="psm")
                        for c in range(4):
                            nc.tensor.matmul(
                                psy[:],
                                oT_t[b * 4 + c][:, t * 128:(t + 1) * 128],
                                wp_t[c][:], start=(c == 0), stop=(c == 3))
                        yt = YSB.tile([128, DIM], bf16, tag="y", name="y")
                        nc.vector.tensor_tensor(yt[:], psy[:], bpb_t[:], ADD)
                        nc.sync.dma_start(
                            out=y[b * N + t * 128: b * N + (t + 1) * 128, :],
                            in_=yt[:])
    nc.compile()
    return nc


def _prep_host(x, Hstack, hop_logits_attn, rel_alpha, Wqkv, Wproj, bproj):
    """Build the GLOBAL (concatenated-over-cores) input arrays directly.

    Shard layouts are chosen so that concatenating per-core shards along
    axis 0 reproduces the flat array itself - zero np.concatenate cost.
    Returns dict name -> global array of shape [8*per_core_rows, cols].
    """
    bf = ml_dtypes.bfloat16
    lg = hop_logits_attn - hop_logits_attn.max(-1, keepdims=True)
    w = np.exp(lg)
    w /= w.sum(-1, keepdims=True)                      # [H, KH]
    # Hstack ships as uint8 (values in [0,1], quantization err ~ bf16's);
    # the 1/255 dequant scale is folded into the scaled identities.
    c_hk = (rel_alpha[:, None] * w).astype(np.float32) / 255.0  # [H, KH]
    eye = np.eye(128, dtype=np.float32)
    ceye = (c_hk.reshape(H * KH, 1, 1) * eye).astype(bf).reshape(CER, 128)
    hts = (Hstack.transpose(0, 2, 1) * 255.0 + 0.5).astype(np.uint8).reshape(HTR, N)
    wqkvT = np.ascontiguousarray(Wqkv.T).astype(np.float32)
    wqkvT[:, :DIM] *= SCALE                            # fold q scaling
    wqkvT = wqkvT.astype(bf)
    wprojT = np.ascontiguousarray(Wproj.T).astype(bf)
    u8row = lambda a: np.ascontiguousarray(a).view(np.uint8).reshape(-1, 1024)
    shr = np.concatenate([
        u8row(wqkvT), u8row(wprojT), u8row(ceye), u8row(eye.astype(bf)),
        u8row(bproj.astype(bf)[None, :]), hts,
        np.zeros((SHR_ROWS - R_HTS - HTR, 1024), np.uint8),
    ], axis=0)
    return {
        "xn": x.reshape(NCORES * TOK, DIM).astype(ml_dtypes.float8_e4m3),
        "shr_in": shr,
    }


def _make_runner(nc):
    """Persistent-jit runner for the bass_exec custom call (the axon/PJRT
    path), so warm calls skip tracing and output zero-buffers are created
    on device instead of being shipped from the host."""
    import jax
    import jax.numpy as jnp
    from jax.sharding import Mesh, PartitionSpec, NamedSharding
    from jax.experimental.shard_map import shard_map
    from concourse import mybir
    from concourse.bass2jax import (
        _bass_exec_p, partition_id_tensor, install_neuronx_cc_hook)

    install_neuronx_cc_hook()
    partition_name = nc.partition_id_tensor.name if nc.partition_id_tensor else None
    in_names, out_names, out_avals = [], [], []
    for alloc in nc.m.functions[0].allocations:
        if not isinstance(alloc, mybir.MemoryLocationSet):
            continue
        name = alloc.memorylocations[0].name
        if alloc.kind == "ExternalInput":
            if name != partition_name:
                in_names.append(name)
        elif alloc.kind == "ExternalOutput":
            out_names.append(name)
            out_avals.append(jax.core.ShapedArray(
                tuple(alloc.tensor_shape), mybir.dt.np(alloc.dtype)))
    n_params = len(in_names)
    n_outs = len(out_avals)
    all_names = in_names + out_names
    if partition_name is not None:
        all_names = all_names + [partition_name]
    donate = tuple(range(n_params, n_params + n_outs))

    def _body(*args):
        operands = list(args)
        if partition_name is not None:
            operands.append(partition_id_tensor())
        outs = _bass_exec_p.bind(
            *operands, out_avals=tuple(out_avals), in_names=tuple(all_names),
            out_names=tuple(out_names), lowering_input_output_aliases=(),
            sim_require_finite=True, sim_require_nnan=True, nc=nc)
        return tuple(outs)

    devices = jax.devices()[:NCORES]
    mesh = Mesh(np.asarray(devices), ("core",))
    spec = NamedSharding(mesh, PartitionSpec("core"))
    in_specs = (PartitionSpec("core"),) * (n_params + n_outs)
    out_specs = (PartitionSpec("core"),) * n_outs
    sharded = jax.jit(
        shard_map(_body, mesh=mesh, in_specs=in_specs, out_specs=out_specs,
                  check_rep=False),
        donate_argnums=donate, keep_unused=True)

    zero_shapes = [(NCORES * a.shape[0], *a.shape[1:]) for a in out_avals]
    zero_dtypes = [a.dtype for a in out_avals]
    zeros_fn = jax.jit(
        lambda: tuple(jnp.zeros(s, d) for s, d in zip(zero_shapes, zero_dtypes)),
        out_shardings=tuple(spec for _ in out_avals))

    prev = []

    def run(global_in: dict):
        ins = [global_in[name] for name in in_names]
        # donate the previous call's (already fetched) output buffers as the
        # custom call's result allocation; first call builds zeros on device
        zs = tuple(prev) if prev else zeros_fn()
        prev.clear()
        outs = sharded(*ins, *zs)
        res = {name: np.asarray(o) for name, o in zip(out_names, outs)}
        prev.extend(outs)
        return res

    return run


def kernel(**inputs):
    if "run" not in _CACHE:
        _CACHE["nc"] = _build()
        _CACHE["run"] = _make_runner(_CACHE["nc"])
    gin = _prep_host(
        np.asarray(inputs["x"], np.float32),
        np.asarray(inputs["Hstack"], np.float32),
        np.asarray(inputs["hop_logits_attn"], np.float32),
        np.asarray(inputs["rel_alpha"], np.float32),
        np.asarray(inputs["Wqkv"], np.float32),
        np.asarray(inputs["Wproj"], np.float32),
        np.asarray(inputs["bproj"], np.float32))
    outs = _CACHE["run"](gin)
    return outs["y"].astype(np.float32).reshape(B, N, DIM)
